# revision 7
# baseline (speedup 1.0000x reference)
"""BinaryLinear kernel for Trainium2, data-parallel over 8 NeuronCores.

Computes y = x @ (sign(W) * scale).T + b where
  sign(w) = +1 if w >= 0 else -1
  scale_o = max(mean_i |W[o,i]|, 1e-6)           (per output row)

Strategy
--------
- Shard batch (32768) across 8 cores -> 4096 rows/core; replicate weights.
- sign(W) and scale are computed on the HOST (scale from full-precision W,
  so that error source is gone entirely); the device only does matmuls and
  a fused scale*psum+bias epilogue.
- Mixed precision split of the 1024-long contraction, chosen so the
  measured max-rel error (1.79e-2) stays under the 2e-2 gate:
    k in [0,512):   x in fp8 e4m3, sign in fp8, matmul in DoubleRow perf
                    mode (2 fp8 weights per PE cell, rhs pair consumed at
                    2 fp8/partition/cycle -> K=256 per 512-cycle MM)
    k in [512,1024): x in bf16, sign in fp8 (exact +-1; mixed-dtype
                    lhsT fp8 x rhs bf16 runs at the full bf16 rate)
  Per (batch-block, out-block) PSUM group: 2 DR MMs + 4 bf16 MMs; the
  trace-measured steady state is a 663 ns period per {DR, bf16, bf16}
  triple = ~96.5% PE-array occupancy, i.e. the stream is at the
  accuracy-constrained PE floor (int8, which would beat e4m3 error at
  the same DR rate, is rejected by the BIR verifier; e3m4 has no
  DoubleRow; more fp8 columns breaks the 2e-2 gate at 2.5e-2).
- Block 0 runs its two DoubleRow c-sweeps FIRST (phase A: 16 DR MMs
  needing only sign+x8 fp8 tiles, 512KB of DMA), then the bf16 units in
  skewed waves (phase B).  The PE engine queue is strict FIFO, so
  without this the first bf16 MM - whose wt/xb tiles land ~4us after
  the fp8 head - stalls the whole stream behind it.  Phase A is a
  ~4us runway that covers the bf16 head DMA and keeps the PE busy
  through the HAM clock-gate window (continuous activity from the
  warmups -> K=8/8 early, no mid-stream re-throttle).
- Blocks 1..7 use the skewed wave schedule: MM(unit u, out-chunk c) at
  wave u+c, so the 8 PSUM banks finish staggered ~1 wave apart,
  epilogues never queue up, and bank recycling never stalls the PE.
- DMA: per-queue throughput is ~85 GB/s in the head regardless of
  transfer size (descriptor/trigger paced), so the schedule is a
  deadline-balanced assignment of 128-768KB pieces to the three
  DMA-capable queues (sync/scalar/gpsimd) in strict first-need order;
  x8/xb are packed on the host so each (unit-group, stage) piece is a
  contiguous 2KB+-row slab, and scale/bias are host-packed into one
  [128,16] tile (the gather it replaces cost 2.3us of queue time).
- Epilogues alternate DVE / ACT per out-chunk; outputs collect in
  [128, 1024] bf16 tiles (two batch blocks) for full-rate 2KB-per-
  partition stores; the last block's per-c stores fan out over the
  three queues to shorten the kernel tail.
"""

import os
import sys
import types

for _p in ("/opt/trn_rl_repo",):
    if _p not in sys.path and os.path.isdir(_p):
        sys.path.append(_p)

import numpy as np
import ml_dtypes

import concourse.bacc as bacc
import concourse.mybir as mybir
from concourse import tile
from concourse.bass_utils import run_bass_kernel_spmd

N_CORES = 8
BATCH = 32768
SHARD = BATCH // N_CORES          # 4096 rows per core
IN = 1024
OUT = 1024
EPS = 1e-6
P = 128                           # SBUF partitions
NB = 512                          # moving free-dim per matmul
NBC = SHARD // NB                 # 8 batch blocks per core
OC = OUT // P                     # 8 output-feature chunks
K8 = 512                          # contraction columns done in fp8
JP = K8 // (2 * P)                # 2 DoubleRow k-pair units (256 each)
KB = (IN - K8) // P               # 4 bf16 k-chunk units (128 each)
NU = JP + KB                      # 6 accumulation units per group

F32 = mybir.dt.float32
BF16 = mybir.dt.bfloat16
FP8 = mybir.dt.float8e4
Alu = mybir.AluOpType
Act = mybir.ActivationFunctionType
DRMODE = mybir.MatmulPerfMode.DoubleRow

# Dummy matmuls bridge the PE from the preamble (~7us) until the first
# real tiles land (~10us): continuous PE activity into phase A keeps the
# HAM clock-gate busy-window filled so the PE un-throttles early.
WARM_SMALL = 24
WARM_BIG = 2

# batch-block DMA stages (one contiguous slab per stage per tensor group)
X_STAGES = [(0, 1), (1, 2), (2, 5), (5, 8)]


def _install_trace_shim():
    """antenv.axon_hooks is absent in this image; recreate it so
    run_bass_kernel_spmd(trace=True) can capture NTFF profiles."""
    try:
        import antenv.axon_hooks  # noqa: F401
        return
    except ImportError:
        pass
    try:
        import trn_agent_boot.trn_boot as tb
        hooks = types.ModuleType("antenv.axon_hooks")
        hooks._hook = tb._ntff_profile_via_ctypes("/opt/axon/libaxon_pjrt.so")
        hooks.get_axon_ntff_profile_hook = lambda: hooks._hook
        hooks.set_axon_ntff_profile_hook = lambda h: setattr(hooks, "_hook", h)
        sys.modules["antenv.axon_hooks"] = hooks
        import concourse.bass_utils as bass_utils
        bass_utils.upload_artifacts = lambda tmpdir: f"file://{tmpdir}"
    except Exception:
        pass


def build_program():
    nc = bacc.Bacc("TRN2", target_bir_lowering=False, debug=False,
                   num_devices=N_CORES)

    # x8: fp8 part of x^T packed block-major: row p, block nb at byte
    # nb*2048, layout [j][i][nn] inside -> per-stage contiguous slabs,
    # rhs slices are [128, 2, 512] DoubleRow APs.
    x8_d = nc.dram_tensor("x8", [P, NBC * 2048], FP8, kind="ExternalInput")
    # xb: bf16 part of x^T packed [p][mp][nb][mm][nn] -> per-(mp, stage)
    # contiguous slabs with 2KB+ rows.
    xb_d = nc.dram_tensor("xb", [P, 2 * NBC * 1024], BF16,
                          kind="ExternalInput")
    # st: fp8 sign(W)^T for k<512, row j*128+p, cols [oh][i][o']
    st_d = nc.dram_tensor("st", [JP * P, 2 * OUT], FP8, kind="ExternalInput")
    # wt: fp8 sign(W)^T for k>=512 (+-1 exact in e4m3; moving rhs stays
    # bf16 so the matmul runs at the 1-column/cycle bf16 rate)
    wt_d = nc.dram_tensor("wt", [KB * P, OUT], FP8, kind="ExternalInput")
    # scb: host-packed scale/bias columns [p, c] / [p, OC+c]
    scb_d = nc.dram_tensor("scb", [P, 2 * OC], F32, kind="ExternalInput")
    yt_d = nc.dram_tensor("yt", [OUT, SHARD], BF16, kind="ExternalOutput")

    with tile.TileContext(nc) as tc:
        with (
            tc.tile_pool(name="w_pool", bufs=1) as w_pool,
            tc.tile_pool(name="x_pool", bufs=1) as x_pool,
            tc.tile_pool(name="misc", bufs=1) as misc,
            tc.tile_pool(name="ps", bufs=8, space="PSUM") as ps_pool,
            tc.tile_pool(name="yo_pool", bufs=8) as yo_pool,
        ):
            # ---- PE warm-up (no input deps)
            warm = misc.tile([P, NB], BF16, tag="warm", name="warm")
            nc.vector.memset(warm[:], 0.0)
            wps = ps_pool.tile([P, NB], F32, tag="ps", name="wps")
            for _ in range(WARM_SMALL):
                nc.tensor.matmul(wps[:, 0:64], warm[:, 0:P], warm[:, 0:64],
                                 start=True, stop=True)
            for _ in range(WARM_BIG):
                nc.tensor.matmul(wps[:], warm[:, 0:P], warm[:],
                                 start=True, stop=True)

            # ---- tiles
            st8 = [[w_pool.tile([P, 2, OUT // 2], FP8, tag=f"st{j}_{oh}",
                                name=f"st{j}_{oh}") for oh in range(2)]
                   for j in range(JP)]
            wt = [w_pool.tile([P, OUT], FP8, tag=f"wt{m}", name=f"wt{m}")
                  for m in range(KB)]
            # x8 stage 0 split per j (phase A consumes j0 first); stages
            # >=1 are single [P, (b1-b0)*2048] slabs.
            x8s0 = [x_pool.tile([P, 2 * NB], FP8, tag=f"x80_{j}",
                                name=f"x80_{j}") for j in range(JP)]
            x8s = [None] + [
                x_pool.tile([P, (b1 - b0) * 2048], FP8,
                            tag=f"x8s_{si}", name=f"x8s_{si}")
                for si, (b0, b1) in enumerate(X_STAGES) if si >= 1]
            # xb per (mp=unit-pair, stage) contiguous slabs
            xbp = [[x_pool.tile([P, (b1 - b0) * 1024], BF16,
                                tag=f"xb{mp}_{si}", name=f"xb{mp}_{si}")
                    for si, (b0, b1) in enumerate(X_STAGES)]
                   for mp in range(KB // 2)]
            scb = misc.tile([P, 2 * OC], F32, tag="scb", name="scb")

            def load_x8s(si, eng):
                b0, b1 = X_STAGES[si]
                eng.dma_start(x8s[si][:], x8_d.ap()[:, b0 * 2048:b1 * 2048])

            def load_xbp(mp, si, eng):
                b0, b1 = X_STAGES[si]
                base = mp * NBC * 1024
                eng.dma_start(xbp[mp][si][:],
                              xb_d.ap()[:, base + b0 * 1024:base + b1 * 1024])

            def load_st(j, oh, eng):
                eng.dma_start(st8[j][oh][:],
                              st_d.ap()[j * P:(j + 1) * P,
                                        oh * OUT:(oh + 1) * OUT])

            # ---- input DMAs: deadline-balanced across the three queues,
            # strict first-need order (~85 GB/s per queue from ~8.8us).
            # Phase A needs st+x8(blk0) by ~10-13us; phase B needs wt+
            # xb(blk0) by ~14-16.5us; block1 x8 by ~18.7, xb by ~19.5-21;
            # block2+ slabs are leisurely.
            load_st(0, 0, nc.sync)                    # pA w0   @10.3
            nc.scalar.dma_start(x8s0[0][:], x8_d.ap()[:, 0:1024])
            load_st(1, 0, nc.gpsimd)                  # pA u1   @10.3
            load_st(0, 1, nc.sync)                    # pA w4   @11.8
            nc.scalar.dma_start(x8s0[1][:], x8_d.ap()[:, 1024:2048])
            load_xbp(0, 0, nc.gpsimd)                 # pB m0/m1 @13.3
            load_st(1, 1, nc.sync)                    # pA u1 w4 @13.3
            nc.scalar.dma_start(wt[0][:], wt_d.ap()[0:P, :])       # @13.3
            nc.gpsimd.dma_start(scb[:], scb_d.ap())   # first epi @13.4
            nc.scalar.dma_start(wt[1][:], wt_d.ap()[P:2 * P, :])   # @14.8
            load_xbp(1, 0, nc.sync)                   # pB m2/m3 @16.3
            load_x8s(1, nc.gpsimd)                    # blk1 pA  @16.4
            nc.scalar.dma_start(wt[2][:], wt_d.ap()[2 * P:3 * P, :])
            nc.scalar.dma_start(wt[3][:], wt_d.ap()[3 * P:4 * P, :])
            load_xbp(0, 1, nc.sync)                   # blk1 pB  @19.3
            load_xbp(1, 1, nc.scalar)                 # blk1 pB  @20.8
            load_x8s(2, nc.gpsimd)                    # blk2-4   @25.4
            load_xbp(0, 2, nc.sync)                   # blk2-4   @28.3
            load_xbp(1, 2, nc.scalar)                 # blk2-4   @29.8
            load_x8s(3, nc.gpsimd)                    # blk5-7   @34.4
            load_xbp(0, 3, nc.sync)                   # blk5-7   @37.3
            load_xbp(1, 3, nc.scalar)                 # blk5-7   @38.8

            def stage_of(n):
                for si, (b0, b1) in enumerate(X_STAGES):
                    if b0 <= n < b1:
                        return si, n - b0
                raise AssertionError(n)

            def rhs_for(u, n):
                si, ln = stage_of(n)
                if u < JP:
                    if si == 0:
                        return x8s0[u][:].rearrange("p (i n) -> p i n", i=2)
                    base = ln * 2048 + u * 1024
                    return x8s[si][:, base:base + 1024].rearrange(
                        "p (i n) -> p i n", i=2)
                m = u - JP
                mp, mm = m // 2, m % 2
                base = ln * 1024 + mm * NB
                return xbp[mp][si][:, base:base + NB]

            yo_cur = [None] * OC

            def epilogue(n, c, ps):
                half = n % 2
                if half == 0:
                    yo_cur[c] = yo_pool.tile([P, 2 * NB], BF16, tag="yo",
                                             name=f"yo{n}_{c}")
                yo = yo_cur[c]
                dst = yo[:, half * NB:(half + 1) * NB]
                if c % 2 == 0:
                    nc.vector.tensor_scalar(dst, ps[:], scb[:, c:c + 1],
                                            scb[:, OC + c:OC + c + 1],
                                            Alu.mult, Alu.add)
                else:
                    nc.scalar.activation(dst, ps[:], Act.Identity,
                                         bias=scb[:, OC + c:OC + c + 1],
                                         scale=scb[:, c:c + 1])
                if n == NBC - 2:
                    # penultimate block: store its half immediately so it
                    # overlaps the last block's compute
                    nc.scalar.dma_start(
                        yt_d.ap()[c * P:(c + 1) * P, n * NB:(n + 1) * NB],
                        yo[:, 0:NB])
                elif n == NBC - 1:
                    # last block: per-c half stores fan out over the three
                    # DMA queues as each staggered epilogue completes ->
                    # short kernel tail
                    eng = (nc.sync, nc.scalar, nc.gpsimd)[c % 3]
                    eng.dma_start(
                        yt_d.ap()[c * P:(c + 1) * P, n * NB:(n + 1) * NB],
                        yo[:, NB:2 * NB])
                elif half == 1:
                    eng = nc.scalar if c % 2 == 1 else nc.sync
                    eng.dma_start(
                        yt_d.ap()[c * P:(c + 1) * P,
                                  (n - 1) * NB:(n + 1) * NB],
                        yo[:])

            def lhsT_dr(u, c):
                return st8[u][c // 4][:, :, (c % 4) * P:(c % 4 + 1) * P]

            # Per-bank unit order for blocks >=1: DoubleRow MMs at slots 0
            # and 3 so two DR MMs are never issued back-to-back (a DR pair
            # costs an extra ~30ns drain gap when adjacent).
            UORDER = (0, 2, 3, 1, 4, 5)

            def mm(s, c, n, ps):
                u = UORDER[s]
                if u < JP:
                    nc.tensor.matmul(ps[:], lhsT_dr(u, c),
                                     rhs_for(u, n), start=(s == 0), stop=False,
                                     perf_mode=DRMODE)
                else:
                    nc.tensor.matmul(ps[:], wt[u - JP][:, c * P:(c + 1) * P],
                                     rhs_for(u, n), start=(s == 0),
                                     stop=(s == NU - 1))

            # ---- block 0: phase A (DR c-sweeps, fp8 data only) then
            # phase B (bf16 units in skewed waves)
            yps = [ps_pool.tile([P, NB], F32, tag="ps", name=f"yp0_{c}")
                   for c in range(OC)]
            for u in range(JP):
                for c in range(OC):
                    nc.tensor.matmul(yps[c][:], lhsT_dr(u, c), rhs_for(u, 0),
                                     start=(u == 0), stop=False,
                                     perf_mode=DRMODE)
            for wv in range(KB + OC - 1):
                for c in range(OC):
                    s = wv - c
                    if 0 <= s < KB:
                        nc.tensor.matmul(
                            yps[c][:], wt[s][:, c * P:(c + 1) * P],
                            rhs_for(JP + s, 0), start=False,
                            stop=(s == KB - 1))
                        if s == KB - 1:
                            epilogue(0, c, yps[c])

            # ---- blocks 1..7: skewed waves.  MM(unit u, out-chunk c) at
            # wave u+c; bank completions stagger ~1 wave apart.
            for n in range(1, NBC):
                yps = [ps_pool.tile([P, NB], F32, tag="ps", name=f"yp{n}_{c}")
                       for c in range(OC)]
                for wv in range(NU + OC - 1):
                    for c in range(OC):
                        u = wv - c
                        if 0 <= u < NU:
                            mm(u, c, n, yps[c])
                            if u == NU - 1:
                                epilogue(n, c, yps[c])

    nc.compile()
    return nc


_NC = None


def _get_program():
    global _NC
    if _NC is None:
        _NC = build_program()
    return _NC


def kernel(x: np.ndarray, W: np.ndarray, b: np.ndarray) -> np.ndarray:
    assert x.shape == (BATCH, IN) and W.shape == (OUT, IN) and b.shape == (OUT,)
    nc = _get_program()

    Wf = np.asarray(W, dtype=np.float32)
    sgnT = np.where(Wf >= 0, np.float32(1.0), np.float32(-1.0)).T  # [in, out]
    # st cols per j are [oh (out half)][i (k subtile)][o']
    st_pack = np.ascontiguousarray(
        sgnT[:K8].reshape(JP, 2, P, 2, OUT // 2).transpose(0, 2, 3, 1, 4)
        .reshape(JP * P, 2 * OUT)).astype(ml_dtypes.float8_e4m3)
    wt_pack = np.ascontiguousarray(sgnT[K8:]).astype(ml_dtypes.float8_e4m3)
    sc = np.maximum(np.abs(Wf).mean(axis=1), EPS).astype(np.float32)
    b32 = np.asarray(b, dtype=np.float32)
    # scb[p, c] = sc[c*128+p]; scb[p, OC+c] = b[c*128+p]
    scb = np.ascontiguousarray(
        np.concatenate([sc.reshape(OC, P).T, b32.reshape(OC, P).T],
                       axis=1).astype(np.float32))

    in_maps = []
    for c in range(N_CORES):
        xt = x[c * SHARD:(c + 1) * SHARD].T      # [in, n] view
        # x8 block-major: (j,i,p,nb,nn) -> (p, nb, j, i, nn)
        x8 = xt[:K8].astype(ml_dtypes.float8_e4m3)
        x8 = np.ascontiguousarray(
            x8.reshape(JP, 2, P, NBC, NB).transpose(2, 3, 0, 1, 4)
            .reshape(P, NBC * 2048))
        # xb: (mp,mm,p,nb,nn) -> (p, mp, nb, mm, nn)
        xb = xt[K8:].astype(ml_dtypes.bfloat16)
        xb = np.ascontiguousarray(
            xb.reshape(2, 2, P, NBC, NB).transpose(2, 0, 3, 1, 4)
            .reshape(P, 2 * NBC * 1024))
        in_maps.append({"x8": x8, "xb": xb, "st": st_pack, "wt": wt_pack,
                        "scb": scb})

    trace = bool(int(os.environ.get("BINLIN_TRACE", "0")))
    if trace:
        _install_trace_shim()
    res = run_bass_kernel_spmd(nc, in_maps, core_ids=list(range(N_CORES)),
                               trace=trace)
    if trace and res.exec_time_ns is not None:
        print(f"HW exec time: {res.exec_time_ns} ns", flush=True)

    y = np.empty((BATCH, OUT), dtype=np.float32)
    for c in range(N_CORES):
        y[c * SHARD:(c + 1) * SHARD] = res.results[c]["yt"].T.astype(np.float32)
    return y


# revision 9
# speedup vs baseline: 1.0924x; 1.0924x over previous
"""BinaryLinear kernel for Trainium2, data-parallel over 8 NeuronCores.

Computes y = x @ (sign(W) * scale).T + b where
  sign(w) = +1 if w >= 0 else -1
  scale_o = max(mean_i |W[o,i]|, 1e-6)           (per output row)

Strategy
--------
- Shard batch (32768) across 8 cores -> 4096 rows/core; replicate weights.
- sign(W) and scale are computed on the HOST (scale from full-precision W,
  so that error source is gone entirely); the device only does matmuls and
  a fused scale*psum+bias epilogue.
- Mixed precision split of the 1024-long contraction, chosen so the
  measured max-rel error (1.79e-2) stays under the 2e-2 gate:
    k in [0,512):   x in fp8 e4m3, sign in fp8, matmul in DoubleRow perf
                    mode (2 fp8 weights per PE cell, rhs pair consumed at
                    2 fp8/partition/cycle -> K=256 per 512-cycle MM)
    k in [512,1024): x in bf16, sign in fp8 (exact +-1; mixed-dtype
                    lhsT fp8 x rhs bf16 runs at the full bf16 rate)
  Per (batch-block, out-block) PSUM group: 2 DR MMs + 4 bf16 MMs; the
  trace-measured steady state is a 663 ns period per {DR, bf16, bf16}
  triple = ~96.5% PE-array occupancy, i.e. the stream is at the
  accuracy-constrained PE floor (int8, which would beat e4m3 error at
  the same DR rate, is rejected by the BIR verifier; e3m4 has no
  DoubleRow; more fp8 columns breaks the 2e-2 gate at 2.5e-2).
- Block 0 runs its two DoubleRow c-sweeps FIRST (phase A: 16 DR MMs
  needing only sign+x8 fp8 tiles, 512KB of DMA), then the bf16 units in
  skewed waves (phase B).  The PE engine queue is strict FIFO, so
  without this the first bf16 MM - whose wt/xb tiles land ~4us after
  the fp8 head - stalls the whole stream behind it.  Phase A is a
  ~4us runway that covers the bf16 head DMA and keeps the PE busy
  through the HAM clock-gate window (continuous activity from the
  warmups -> K=8/8 early, no mid-stream re-throttle).
- Blocks 1..7 use the skewed wave schedule: MM(unit u, out-chunk c) at
  wave u+c, so the 8 PSUM banks finish staggered ~1 wave apart,
  epilogues never queue up, and bank recycling never stalls the PE.
- DMA: per-queue throughput is ~85 GB/s in the head regardless of
  transfer size (descriptor/trigger paced), so the schedule is a
  deadline-balanced assignment of 128-768KB pieces to the three
  DMA-capable queues (sync/scalar/gpsimd) in strict first-need order;
  x8/xb are packed on the host so each (unit-group, stage) piece is a
  contiguous 2KB+-row slab, and scale/bias are host-packed into one
  [128,16] tile (the gather it replaces cost 2.3us of queue time).
- Epilogues alternate DVE / ACT per out-chunk; outputs collect in
  [128, 1024] bf16 tiles (two batch blocks) for full-rate 2KB-per-
  partition stores; the last block's per-c stores fan out over the
  three queues to shorten the kernel tail.
"""

import os
import sys
import types

for _p in ("/opt/trn_rl_repo",):
    if _p not in sys.path and os.path.isdir(_p):
        sys.path.append(_p)

import numpy as np
import ml_dtypes

import concourse.bacc as bacc
import concourse.mybir as mybir
from concourse import tile
from concourse.bass_utils import run_bass_kernel_spmd

N_CORES = 8
BATCH = 32768
SHARD = BATCH // N_CORES          # 4096 rows per core
IN = 1024
OUT = 1024
EPS = 1e-6
P = 128                           # SBUF partitions
NB = 512                          # moving free-dim per matmul
NBC = SHARD // NB                 # 8 batch blocks per core
OC = OUT // P                     # 8 output-feature chunks
K8 = 512                          # contraction columns done in fp8
JP = K8 // (2 * P)                # 2 DoubleRow k-pair units (256 each)
KB = (IN - K8) // P               # 4 bf16 k-chunk units (128 each)
NU = JP + KB                      # 6 accumulation units per group

F32 = mybir.dt.float32
BF16 = mybir.dt.bfloat16
FP8 = mybir.dt.float8e4
Alu = mybir.AluOpType
Act = mybir.ActivationFunctionType
DRMODE = mybir.MatmulPerfMode.DoubleRow

# Dummy matmuls bridge the PE from the preamble (~7us) until the first
# real tiles land (~10us): continuous PE activity into phase A keeps the
# HAM clock-gate busy-window filled so the PE un-throttles early.
WARM_SMALL = 24
WARM_BIG = 2

# batch-block DMA stages (one contiguous slab per stage per tensor group)
X_STAGES = [(0, 1), (1, 2), (2, 5), (5, 8)]


def _install_trace_shim():
    """antenv.axon_hooks is absent in this image; recreate it so
    run_bass_kernel_spmd(trace=True) can capture NTFF profiles."""
    try:
        import antenv.axon_hooks  # noqa: F401
        return
    except ImportError:
        pass
    try:
        import trn_agent_boot.trn_boot as tb
        hooks = types.ModuleType("antenv.axon_hooks")
        hooks._hook = tb._ntff_profile_via_ctypes("/opt/axon/libaxon_pjrt.so")
        hooks.get_axon_ntff_profile_hook = lambda: hooks._hook
        hooks.set_axon_ntff_profile_hook = lambda h: setattr(hooks, "_hook", h)
        sys.modules["antenv.axon_hooks"] = hooks
        import concourse.bass_utils as bass_utils
        bass_utils.upload_artifacts = lambda tmpdir: f"file://{tmpdir}"
    except Exception:
        pass


def build_program():
    nc = bacc.Bacc("TRN2", target_bir_lowering=False, debug=False,
                   num_devices=N_CORES)

    # x8: fp8 part of x^T packed block-major: row p, block nb at byte
    # nb*2048, layout [j][i][nn] inside -> per-stage contiguous slabs,
    # rhs slices are [128, 2, 512] DoubleRow APs.
    x8_d = nc.dram_tensor("x8", [P, NBC * 2048], FP8, kind="ExternalInput")
    # xb: bf16 part of x^T packed [p][mp][nb][mm][nn] -> per-(mp, stage)
    # contiguous slabs with 2KB+ rows.
    xb_d = nc.dram_tensor("xb", [P, 2 * NBC * 1024], BF16,
                          kind="ExternalInput")
    # st: fp8 sign(W)^T for k<512, row j*128+p, cols [oh][i][o']
    st_d = nc.dram_tensor("st", [JP * P, 2 * OUT], FP8, kind="ExternalInput")
    # wt: fp8 sign(W)^T for k>=512 (+-1 exact in e4m3; moving rhs stays
    # bf16 so the matmul runs at the 1-column/cycle bf16 rate)
    wt_d = nc.dram_tensor("wt", [KB * P, OUT], FP8, kind="ExternalInput")
    # scb: host-packed scale/bias columns [p, c] / [p, OC+c]
    scb_d = nc.dram_tensor("scb", [P, 2 * OC], F32, kind="ExternalInput")
    yt_d = nc.dram_tensor("yt", [OUT, SHARD], BF16, kind="ExternalOutput")

    with tile.TileContext(nc) as tc:
        with (
            tc.tile_pool(name="w_pool", bufs=1) as w_pool,
            tc.tile_pool(name="x_pool", bufs=1) as x_pool,
            tc.tile_pool(name="misc", bufs=1) as misc,
            tc.tile_pool(name="ps", bufs=8, space="PSUM") as ps_pool,
            tc.tile_pool(name="yo_pool", bufs=8) as yo_pool,
        ):
            # ---- PE warm-up (no input deps)
            warm = misc.tile([P, NB], BF16, tag="warm", name="warm")
            nc.vector.memset(warm[:], 0.0)
            wps = ps_pool.tile([P, NB], F32, tag="ps", name="wps")
            for _ in range(WARM_SMALL):
                nc.tensor.matmul(wps[:, 0:64], warm[:, 0:P], warm[:, 0:64],
                                 start=True, stop=True)
            for _ in range(WARM_BIG):
                nc.tensor.matmul(wps[:], warm[:, 0:P], warm[:],
                                 start=True, stop=True)

            # ---- tiles
            st8 = [[w_pool.tile([P, 2, OUT // 2], FP8, tag=f"st{j}_{oh}",
                                name=f"st{j}_{oh}") for oh in range(2)]
                   for j in range(JP)]
            wt = [w_pool.tile([P, OUT], FP8, tag=f"wt{m}", name=f"wt{m}")
                  for m in range(KB)]
            # x8 stage 0 split per j (phase A consumes j0 first); stages
            # >=1 are single [P, (b1-b0)*2048] slabs.
            x8s0 = [x_pool.tile([P, 2 * NB], FP8, tag=f"x80_{j}",
                                name=f"x80_{j}") for j in range(JP)]
            x8s = [None] + [
                x_pool.tile([P, (b1 - b0) * 2048], FP8,
                            tag=f"x8s_{si}", name=f"x8s_{si}")
                for si, (b0, b1) in enumerate(X_STAGES) if si >= 1]
            # xb per (mp=unit-pair, stage) contiguous slabs
            xbp = [[x_pool.tile([P, (b1 - b0) * 1024], BF16,
                                tag=f"xb{mp}_{si}", name=f"xb{mp}_{si}")
                    for si, (b0, b1) in enumerate(X_STAGES)]
                   for mp in range(KB // 2)]
            scb = misc.tile([P, 2 * OC], F32, tag="scb", name="scb")

            def load_x8s(si, eng):
                b0, b1 = X_STAGES[si]
                eng.dma_start(x8s[si][:], x8_d.ap()[:, b0 * 2048:b1 * 2048])

            def load_xbp(mp, si, eng):
                b0, b1 = X_STAGES[si]
                base = mp * NBC * 1024
                eng.dma_start(xbp[mp][si][:],
                              xb_d.ap()[:, base + b0 * 1024:base + b1 * 1024])

            def load_st(j, oh, eng):
                eng.dma_start(st8[j][oh][:],
                              st_d.ap()[j * P:(j + 1) * P,
                                        oh * OUT:(oh + 1) * OUT])

            # ---- input DMAs across the three queues, strict first-need
            # order per queue.  Block 0 runs unit SWEEPS (u0..u5, 8 c's
            # each), so the deadlines are staggered ~1.7us apart and even
            # a 2x-slow queue cannot stall the PE:
            #   pA: st(0,*)+x8j0 @~10; st(1,*)+x8j1 @~12-14
            #   pB: wt_s+xb_m_s @~14.5 + 1.7*s;  blk1 @~21;  blk2+ @~28+
            load_st(0, 0, nc.sync)                    # pA u0 c0-3
            nc.scalar.dma_start(x8s0[0][:], x8_d.ap()[:, 0:1024])
            load_st(1, 0, nc.gpsimd)                  # pA u1 c0-3
            load_st(0, 1, nc.sync)                    # pA u0 c4-7
            nc.scalar.dma_start(x8s0[1][:], x8_d.ap()[:, 1024:2048])
            load_xbp(0, 0, nc.gpsimd)                 # pB m0/m1
            nc.sync.dma_start(wt[0][:], wt_d.ap()[0:P, :])      # pB s0
            load_st(1, 1, nc.scalar)                  # pA u1 c4-7
            nc.gpsimd.dma_start(scb[:], scb_d.ap())   # first epi ~20
            nc.gpsimd.dma_start(wt[1][:], wt_d.ap()[P:2 * P, :])  # pB s1
            load_xbp(1, 0, nc.sync)                   # pB m2/m3
            nc.scalar.dma_start(wt[2][:], wt_d.ap()[2 * P:3 * P, :])
            nc.gpsimd.dma_start(wt[3][:], wt_d.ap()[3 * P:4 * P, :])
            load_x8s(1, nc.gpsimd)                    # blk1 ~21
            load_xbp(0, 1, nc.sync)                   # blk1 waves
            load_xbp(1, 1, nc.scalar)                 # blk1 waves
            load_x8s(2, nc.gpsimd)                    # blk2-4 ~28
            load_xbp(0, 2, nc.sync)
            load_xbp(1, 2, nc.scalar)
            load_x8s(3, nc.gpsimd)                    # blk5-7
            load_xbp(0, 3, nc.sync)
            load_xbp(1, 3, nc.scalar)

            def stage_of(n):
                for si, (b0, b1) in enumerate(X_STAGES):
                    if b0 <= n < b1:
                        return si, n - b0
                raise AssertionError(n)

            def rhs_for(u, n):
                si, ln = stage_of(n)
                if u < JP:
                    if si == 0:
                        return x8s0[u][:].rearrange("p (i n) -> p i n", i=2)
                    base = ln * 2048 + u * 1024
                    return x8s[si][:, base:base + 1024].rearrange(
                        "p (i n) -> p i n", i=2)
                m = u - JP
                mp, mm = m // 2, m % 2
                base = ln * 1024 + mm * NB
                return xbp[mp][si][:, base:base + NB]

            yo_cur = [None] * OC

            def epilogue(n, c, ps):
                half = n % 2
                if half == 0:
                    yo_cur[c] = yo_pool.tile([P, 2 * NB], BF16, tag="yo",
                                             name=f"yo{n}_{c}")
                yo = yo_cur[c]
                dst = yo[:, half * NB:(half + 1) * NB]
                if c % 2 == 0:
                    nc.vector.tensor_scalar(dst, ps[:], scb[:, c:c + 1],
                                            scb[:, OC + c:OC + c + 1],
                                            Alu.mult, Alu.add)
                else:
                    nc.scalar.activation(dst, ps[:], Act.Identity,
                                         bias=scb[:, OC + c:OC + c + 1],
                                         scale=scb[:, c:c + 1])
                if n == NBC - 2:
                    # penultimate block: store its half immediately so it
                    # overlaps the last block's compute
                    nc.scalar.dma_start(
                        yt_d.ap()[c * P:(c + 1) * P, n * NB:(n + 1) * NB],
                        yo[:, 0:NB])
                elif n == NBC - 1:
                    # last block: per-c half stores fan out over the three
                    # DMA queues as each staggered epilogue completes ->
                    # short kernel tail
                    eng = (nc.sync, nc.scalar, nc.gpsimd)[c % 3]
                    eng.dma_start(
                        yt_d.ap()[c * P:(c + 1) * P, n * NB:(n + 1) * NB],
                        yo[:, NB:2 * NB])
                elif half == 1:
                    eng = nc.scalar if c % 2 == 1 else nc.sync
                    eng.dma_start(
                        yt_d.ap()[c * P:(c + 1) * P,
                                  (n - 1) * NB:(n + 1) * NB],
                        yo[:])

            def lhsT_dr(u, c):
                return st8[u][c // 4][:, :, (c % 4) * P:(c % 4 + 1) * P]

            # Per-bank unit order for blocks >=1: DoubleRow MMs at slots 0
            # and 3 so two DR MMs are never issued back-to-back (a DR pair
            # costs an extra ~30ns drain gap when adjacent).
            UORDER = (0, 2, 3, 1, 4, 5)

            def mm(s, c, n, ps):
                u = UORDER[s]
                if u < JP:
                    nc.tensor.matmul(ps[:], lhsT_dr(u, c),
                                     rhs_for(u, n), start=(s == 0), stop=False,
                                     perf_mode=DRMODE)
                else:
                    nc.tensor.matmul(ps[:], wt[u - JP][:, c * P:(c + 1) * P],
                                     rhs_for(u, n), start=(s == 0),
                                     stop=(s == NU - 1))

            # ---- block 0: unit sweeps.  Phase A: DR c-sweeps needing only
            # sign+x8 fp8 data; phase B: bf16 unit c-sweeps whose (wt_s,
            # xb_m_s) tiles are only needed ~1.7us apart, so the head DMA
            # has large slack on every deadline.  Bank c completes at
            # position c of the final sweep (213ns stagger); epilogues
            # alternate DVE/ACT so each engine drains 4 back-to-back.
            yps = [ps_pool.tile([P, NB], F32, tag="ps", name=f"yp0_{c}")
                   for c in range(OC)]
            for u in range(JP):
                for c in range(OC):
                    nc.tensor.matmul(yps[c][:], lhsT_dr(u, c), rhs_for(u, 0),
                                     start=(u == 0), stop=False,
                                     perf_mode=DRMODE)
            for s in range(KB):
                for c in range(OC):
                    nc.tensor.matmul(
                        yps[c][:], wt[s][:, c * P:(c + 1) * P],
                        rhs_for(JP + s, 0), start=False,
                        stop=(s == KB - 1))
                    if s == KB - 1:
                        epilogue(0, c, yps[c])

            # ---- blocks 1..7: skewed waves.  MM(unit u, out-chunk c) at
            # wave u+c; bank completions stagger ~1 wave apart.
            for n in range(1, NBC):
                yps = [ps_pool.tile([P, NB], F32, tag="ps", name=f"yp{n}_{c}")
                       for c in range(OC)]
                for wv in range(NU + OC - 1):
                    for c in range(OC):
                        u = wv - c
                        if 0 <= u < NU:
                            mm(u, c, n, yps[c])
                            if u == NU - 1:
                                epilogue(n, c, yps[c])

    nc.compile()
    return nc


_NC = None


def _get_program():
    global _NC
    if _NC is None:
        _NC = build_program()
    return _NC


def kernel(x: np.ndarray, W: np.ndarray, b: np.ndarray) -> np.ndarray:
    assert x.shape == (BATCH, IN) and W.shape == (OUT, IN) and b.shape == (OUT,)
    nc = _get_program()

    Wf = np.asarray(W, dtype=np.float32)
    sgnT = np.where(Wf >= 0, np.float32(1.0), np.float32(-1.0)).T  # [in, out]
    # st cols per j are [oh (out half)][i (k subtile)][o']
    st_pack = np.ascontiguousarray(
        sgnT[:K8].reshape(JP, 2, P, 2, OUT // 2).transpose(0, 2, 3, 1, 4)
        .reshape(JP * P, 2 * OUT)).astype(ml_dtypes.float8_e4m3)
    wt_pack = np.ascontiguousarray(sgnT[K8:]).astype(ml_dtypes.float8_e4m3)
    sc = np.maximum(np.abs(Wf).mean(axis=1), EPS).astype(np.float32)
    b32 = np.asarray(b, dtype=np.float32)
    # scb[p, c] = sc[c*128+p]; scb[p, OC+c] = b[c*128+p]
    scb = np.ascontiguousarray(
        np.concatenate([sc.reshape(OC, P).T, b32.reshape(OC, P).T],
                       axis=1).astype(np.float32))

    in_maps = []
    for c in range(N_CORES):
        xt = x[c * SHARD:(c + 1) * SHARD].T      # [in, n] view
        # x8 block-major: (j,i,p,nb,nn) -> (p, nb, j, i, nn)
        x8 = xt[:K8].astype(ml_dtypes.float8_e4m3)
        x8 = np.ascontiguousarray(
            x8.reshape(JP, 2, P, NBC, NB).transpose(2, 3, 0, 1, 4)
            .reshape(P, NBC * 2048))
        # xb: (mp,mm,p,nb,nn) -> (p, mp, nb, mm, nn)
        xb = xt[K8:].astype(ml_dtypes.bfloat16)
        xb = np.ascontiguousarray(
            xb.reshape(2, 2, P, NBC, NB).transpose(2, 0, 3, 1, 4)
            .reshape(P, 2 * NBC * 1024))
        in_maps.append({"x8": x8, "xb": xb, "st": st_pack, "wt": wt_pack,
                        "scb": scb})

    trace = bool(int(os.environ.get("BINLIN_TRACE", "0")))
    if trace:
        _install_trace_shim()
    res = run_bass_kernel_spmd(nc, in_maps, core_ids=list(range(N_CORES)),
                               trace=trace)
    if trace and res.exec_time_ns is not None:
        print(f"HW exec time: {res.exec_time_ns} ns", flush=True)

    y = np.empty((BATCH, OUT), dtype=np.float32)
    for c in range(N_CORES):
        y[c * SHARD:(c + 1) * SHARD] = res.results[c]["yt"].T.astype(np.float32)
    return y


# revision 15
# speedup vs baseline: 1.0948x; 1.0022x over previous
"""BinaryLinear kernel for Trainium2, data-parallel over 8 NeuronCores.

Computes y = x @ (sign(W) * scale).T + b where
  sign(w) = +1 if w >= 0 else -1
  scale_o = max(mean_i |W[o,i]|, 1e-6)           (per output row)

Strategy
--------
- Shard batch (32768) across 8 cores -> 4096 rows/core; replicate weights.
- sign(W) and scale are computed on the HOST (scale from full-precision W,
  so that error source is gone entirely); the device only does matmuls and
  a fused scale*psum+bias epilogue.
- Mixed precision split of the 1024-long contraction, chosen so the
  measured max-rel error (1.79e-2) stays under the 2e-2 gate:
    k in [0,512):   x in fp8 e4m3, sign in fp8, matmul in DoubleRow perf
                    mode (2 fp8 weights per PE cell, rhs pair consumed at
                    2 fp8/partition/cycle -> K=256 per 512-cycle MM)
    k in [512,1024): x in bf16, sign in fp8 (exact +-1; mixed-dtype
                    lhsT fp8 x rhs bf16 runs at the full bf16 rate)
  Per (batch-block, out-block) PSUM group: 2 DR MMs + 4 bf16 MMs; the
  trace-measured steady state is a 663 ns period per {DR, bf16, bf16}
  triple = ~96.5% PE-array occupancy, i.e. the stream is at the
  accuracy-constrained PE floor (int8, which would beat e4m3 error at
  the same DR rate, is rejected by the BIR verifier; e3m4 has no
  DoubleRow; more fp8 columns breaks the 2e-2 gate at 2.5e-2).
- Block 0 runs its two DoubleRow c-sweeps FIRST (phase A: 16 DR MMs
  needing only sign+x8 fp8 tiles, 512KB of DMA), then the bf16 units in
  skewed waves (phase B).  The PE engine queue is strict FIFO, so
  without this the first bf16 MM - whose wt/xb tiles land ~4us after
  the fp8 head - stalls the whole stream behind it.  Phase A is a
  ~4us runway that covers the bf16 head DMA and keeps the PE busy
  through the HAM clock-gate window (continuous activity from the
  warmups -> K=8/8 early, no mid-stream re-throttle).
- Blocks 1..7 use the skewed wave schedule: MM(unit u, out-chunk c) at
  wave u+c, so the 8 PSUM banks finish staggered ~1 wave apart,
  epilogues never queue up, and bank recycling never stalls the PE.
- DMA: per-queue throughput is ~85 GB/s in the head regardless of
  transfer size (descriptor/trigger paced), so the schedule is a
  deadline-balanced assignment of 128-768KB pieces to the three
  DMA-capable queues (sync/scalar/gpsimd) in strict first-need order;
  x8/xb are packed on the host so each (unit-group, stage) piece is a
  contiguous 2KB+-row slab, and scale/bias are host-packed into one
  [128,16] tile (the gather it replaces cost 2.3us of queue time).
- Epilogues alternate DVE / ACT per out-chunk; outputs collect in
  [128, 1024] bf16 tiles (two batch blocks) for full-rate 2KB-per-
  partition stores; the last block's per-c stores fan out over the
  three queues to shorten the kernel tail.
"""

import os
import sys
import types

for _p in ("/opt/trn_rl_repo",):
    if _p not in sys.path and os.path.isdir(_p):
        sys.path.append(_p)

import numpy as np
import ml_dtypes

import concourse.bacc as bacc
import concourse.mybir as mybir
from concourse import tile
from concourse.bass_utils import run_bass_kernel_spmd

N_CORES = 8
BATCH = 32768
SHARD = BATCH // N_CORES          # 4096 rows per core
IN = 1024
OUT = 1024
EPS = 1e-6
P = 128                           # SBUF partitions
NB = 512                          # moving free-dim per matmul
NBC = SHARD // NB                 # 8 batch blocks per core
OC = OUT // P                     # 8 output-feature chunks
K8 = 512                          # contraction columns done in fp8
JP = K8 // (2 * P)                # 2 DoubleRow k-pair units (256 each)
KB = (IN - K8) // P               # 4 bf16 k-chunk units (128 each)
NU = JP + KB                      # 6 accumulation units per group

F32 = mybir.dt.float32
BF16 = mybir.dt.bfloat16
FP8 = mybir.dt.float8e4
Alu = mybir.AluOpType
Act = mybir.ActivationFunctionType
DRMODE = mybir.MatmulPerfMode.DoubleRow

# Dummy matmuls bridge the PE from the preamble (~7us) until the first
# real tiles land (10.3-12.3us depending on DMA cold-start): continuous
# PE activity into phase A keeps the HAM clock-gate busy-window filled
# so the PE un-throttles during the warmups and the real stream starts
# at 2.4 GHz.  Overshooting costs ~200ns/excess MM; a gap costs ~4us
# (idle window -> re-throttle + half-rate stream), so size for the
# slow-DMA case.
WARM_SMALL = 24
WARM_BIG = 8

# batch-block DMA stages (one contiguous slab per stage per tensor group)
X_STAGES = [(0, 1), (1, 2), (2, 5), (5, 8)]


def _install_trace_shim():
    """antenv.axon_hooks is absent in this image; recreate it so
    run_bass_kernel_spmd(trace=True) can capture NTFF profiles."""
    try:
        import antenv.axon_hooks  # noqa: F401
        return
    except ImportError:
        pass
    try:
        import trn_agent_boot.trn_boot as tb
        hooks = types.ModuleType("antenv.axon_hooks")
        hooks._hook = tb._ntff_profile_via_ctypes("/opt/axon/libaxon_pjrt.so")
        hooks.get_axon_ntff_profile_hook = lambda: hooks._hook
        hooks.set_axon_ntff_profile_hook = lambda h: setattr(hooks, "_hook", h)
        sys.modules["antenv.axon_hooks"] = hooks
        import concourse.bass_utils as bass_utils
        bass_utils.upload_artifacts = lambda tmpdir: f"file://{tmpdir}"
    except Exception:
        pass


def build_program():
    nc = bacc.Bacc("TRN2", target_bir_lowering=False, debug=False,
                   num_devices=N_CORES)

    # x8: fp8 part of x^T packed block-major: row p, block nb at byte
    # nb*2048, layout [j][i][nn] inside -> per-stage contiguous slabs,
    # rhs slices are [128, 2, 512] DoubleRow APs.
    x8_d = nc.dram_tensor("x8", [P, NBC * 2048], FP8, kind="ExternalInput")
    # xb: bf16 part of x^T packed [p][mp][nb][mm][nn] -> per-(mp, stage)
    # contiguous slabs with 2KB+ rows.
    xb_d = nc.dram_tensor("xb", [P, 2 * NBC * 1024], BF16,
                          kind="ExternalInput")
    # st: fp8 sign(W)^T for k<512, row j*128+p, cols [oh][i][o']
    st_d = nc.dram_tensor("st", [JP * P, 2 * OUT], FP8, kind="ExternalInput")
    # wt: fp8 sign(W)^T for k>=512 (+-1 exact in e4m3; moving rhs stays
    # bf16 so the matmul runs at the 1-column/cycle bf16 rate)
    wt_d = nc.dram_tensor("wt", [KB * P, OUT], FP8, kind="ExternalInput")
    # scb: host-packed scale/bias columns [p, c] / [p, OC+c]
    scb_d = nc.dram_tensor("scb", [P, 2 * OC], F32, kind="ExternalInput")
    yt_d = nc.dram_tensor("yt", [OUT, SHARD], BF16, kind="ExternalOutput")

    with tile.TileContext(nc) as tc:
        with (
            tc.tile_pool(name="w_pool", bufs=1) as w_pool,
            tc.tile_pool(name="x_pool", bufs=1) as x_pool,
            tc.tile_pool(name="misc", bufs=1) as misc,
            tc.tile_pool(name="ps", bufs=8, space="PSUM") as ps_pool,
            tc.tile_pool(name="yo_pool", bufs=8) as yo_pool,
        ):
            # ---- PE warm-up (no input deps)
            warm = misc.tile([P, NB], BF16, tag="warm", name="warm")
            nc.vector.memset(warm[:], 0.0)
            wps = ps_pool.tile([P, NB], F32, tag="ps", name="wps")
            for _ in range(WARM_SMALL):
                nc.tensor.matmul(wps[:, 0:64], warm[:, 0:P], warm[:, 0:64],
                                 start=True, stop=True)
            for _ in range(WARM_BIG):
                nc.tensor.matmul(wps[:], warm[:, 0:P], warm[:],
                                 start=True, stop=True)

            # ---- tiles
            st8 = [[w_pool.tile([P, 2, OUT // 2], FP8, tag=f"st{j}_{oh}",
                                name=f"st{j}_{oh}") for oh in range(2)]
                   for j in range(JP)]
            wt = [w_pool.tile([P, OUT], FP8, tag=f"wt{m}", name=f"wt{m}")
                  for m in range(KB)]
            # x8 stage 0 split per j (phase A consumes j0 first); stages
            # >=1 are single [P, (b1-b0)*2048] slabs.
            x8s0 = [x_pool.tile([P, 2 * NB], FP8, tag=f"x80_{j}",
                                name=f"x80_{j}") for j in range(JP)]
            x8s = [None] + [
                x_pool.tile([P, (b1 - b0) * 2048], FP8,
                            tag=f"x8s_{si}", name=f"x8s_{si}")
                for si, (b0, b1) in enumerate(X_STAGES) if si >= 1]
            # xb: single-block stages (0,1) split per unit m (128KB pieces
            # so the three queues share the tight early deadlines); stages
            # 2-3 are per-(mp=unit-pair) contiguous slabs.
            xbm = [[x_pool.tile([P, NB], BF16, tag=f"xbm{m}_{si}",
                                name=f"xbm{m}_{si}") for m in range(KB)]
                   for si in range(2)]
            xbp = [[None, None] + [
                x_pool.tile([P, (b1 - b0) * 1024], BF16,
                            tag=f"xb{mp}_{si}", name=f"xb{mp}_{si}")
                for si, (b0, b1) in enumerate(X_STAGES) if si >= 2]
                for mp in range(KB // 2)]
            scb = misc.tile([P, 2 * OC], F32, tag="scb", name="scb")

            def load_x8s(si, eng):
                b0, b1 = X_STAGES[si]
                eng.dma_start(x8s[si][:], x8_d.ap()[:, b0 * 2048:b1 * 2048])

            def load_xbp(mp, si, eng):
                b0, b1 = X_STAGES[si]
                base = mp * NBC * 1024
                eng.dma_start(xbp[mp][si][:],
                              xb_d.ap()[:, base + b0 * 1024:base + b1 * 1024])

            def load_xbm(m, si, eng):
                b0, _ = X_STAGES[si]
                mp, mm = m // 2, m % 2
                base = mp * NBC * 1024 + b0 * 1024 + mm * NB
                eng.dma_start(xbm[si][m][:], xb_d.ap()[:, base:base + NB])

            def load_st(j, oh, eng):
                eng.dma_start(st8[j][oh][:],
                              st_d.ap()[j * P:(j + 1) * P,
                                        oh * OUT:(oh + 1) * OUT])

            # ---- input DMAs across the three queues, strict first-need
            # order per queue.  Block 0 runs unit SWEEPS (u0..u5, 8 c's
            # each), so the deadlines are staggered ~1.7us apart and even
            # a 2x-slow queue cannot stall the PE:
            #   pA: st(0,*)+x8j0 @~10; st(1,*)+x8j1 @~12-14
            #   pB: wt_s+xb_m_s @~14.5 + 1.7*s;  blk1 @~21;  blk2+ @~28+
            # round-robin in strict need order; verified feasible against
            # every deadline even at a worst-case 70 GB/s per queue
            load_st(0, 0, nc.sync)                    # pA u0 c0-3
            nc.scalar.dma_start(x8s0[0][:], x8_d.ap()[:, 0:1024])
            load_st(1, 0, nc.gpsimd)                  # pA u1 c0-3
            load_st(0, 1, nc.sync)                    # pA u0 c4-7
            nc.scalar.dma_start(x8s0[1][:], x8_d.ap()[:, 1024:2048])
            load_st(1, 1, nc.gpsimd)                  # pA u1 c4-7
            nc.sync.dma_start(wt[0][:], wt_d.ap()[0:P, :])        # pB s0
            load_xbm(0, 0, nc.scalar)                 # pB s0
            nc.gpsimd.dma_start(wt[1][:], wt_d.ap()[P:2 * P, :])  # pB s1
            load_xbm(1, 0, nc.sync)                   # pB s1
            nc.scalar.dma_start(wt[2][:], wt_d.ap()[2 * P:3 * P, :])
            load_xbm(2, 0, nc.gpsimd)                 # pB s2
            nc.sync.dma_start(wt[3][:], wt_d.ap()[3 * P:4 * P, :])
            load_xbm(3, 0, nc.scalar)                 # pB s3
            nc.gpsimd.dma_start(scb[:], scb_d.ap())   # first epi ~22
            load_x8s(1, nc.sync)                      # blk1 u0/u1
            load_xbm(0, 1, nc.scalar)                 # blk1 waves
            load_xbm(1, 1, nc.gpsimd)
            load_xbm(2, 1, nc.scalar)
            load_xbm(3, 1, nc.gpsimd)
            load_x8s(2, nc.gpsimd)                    # blk2-4
            load_xbp(0, 2, nc.sync)
            load_xbp(1, 2, nc.scalar)
            load_x8s(3, nc.gpsimd)                    # blk5-7
            load_xbp(0, 3, nc.sync)
            load_xbp(1, 3, nc.scalar)

            def stage_of(n):
                for si, (b0, b1) in enumerate(X_STAGES):
                    if b0 <= n < b1:
                        return si, n - b0
                raise AssertionError(n)

            def rhs_for(u, n):
                si, ln = stage_of(n)
                if u < JP:
                    if si == 0:
                        return x8s0[u][:].rearrange("p (i n) -> p i n", i=2)
                    base = ln * 2048 + u * 1024
                    return x8s[si][:, base:base + 1024].rearrange(
                        "p (i n) -> p i n", i=2)
                m = u - JP
                if si <= 1:
                    return xbm[si][m][:]
                mp, mm = m // 2, m % 2
                base = ln * 1024 + mm * NB
                return xbp[mp][si][:, base:base + NB]

            yo_cur = [None] * OC

            def epilogue(n, c, ps):
                half = n % 2
                if half == 0:
                    yo_cur[c] = yo_pool.tile([P, 2 * NB], BF16, tag="yo",
                                             name=f"yo{n}_{c}")
                yo = yo_cur[c]
                dst = yo[:, half * NB:(half + 1) * NB]
                if c % 2 == 0:
                    nc.vector.tensor_scalar(dst, ps[:], scb[:, c:c + 1],
                                            scb[:, OC + c:OC + c + 1],
                                            Alu.mult, Alu.add)
                else:
                    nc.scalar.activation(dst, ps[:], Act.Identity,
                                         bias=scb[:, OC + c:OC + c + 1],
                                         scale=scb[:, c:c + 1])
                if n == NBC - 2:
                    # penultimate block: store its half immediately so it
                    # overlaps the last block's compute
                    nc.scalar.dma_start(
                        yt_d.ap()[c * P:(c + 1) * P, n * NB:(n + 1) * NB],
                        yo[:, 0:NB])
                elif n == NBC - 1:
                    # last block: per-c half stores fan out over the three
                    # DMA queues as each staggered epilogue completes ->
                    # short kernel tail
                    eng = (nc.sync, nc.scalar, nc.gpsimd)[c % 3]
                    eng.dma_start(
                        yt_d.ap()[c * P:(c + 1) * P, n * NB:(n + 1) * NB],
                        yo[:, NB:2 * NB])
                elif half == 1:
                    eng = nc.scalar if c % 2 == 1 else nc.sync
                    eng.dma_start(
                        yt_d.ap()[c * P:(c + 1) * P,
                                  (n - 1) * NB:(n + 1) * NB],
                        yo[:])

            def lhsT_dr(u, c):
                return st8[u][c // 4][:, :, (c % 4) * P:(c % 4 + 1) * P]

            # Per-bank unit order for blocks >=1: DoubleRow MMs at slots 0
            # and 3 so two DR MMs are never issued back-to-back (a DR pair
            # costs an extra ~30ns drain gap when adjacent).
            UORDER = (0, 2, 3, 1, 4, 5)

            def mm(s, c, n, ps):
                u = UORDER[s]
                if u < JP:
                    nc.tensor.matmul(ps[:], lhsT_dr(u, c),
                                     rhs_for(u, n), start=(s == 0), stop=False,
                                     perf_mode=DRMODE)
                else:
                    nc.tensor.matmul(ps[:], wt[u - JP][:, c * P:(c + 1) * P],
                                     rhs_for(u, n), start=(s == 0),
                                     stop=(s == NU - 1))

            # ---- block 0: unit sweeps.  Phase A: DR c-sweeps needing only
            # sign+x8 fp8 data; phase B: bf16 unit c-sweeps whose (wt_s,
            # xb_m_s) tiles are only needed ~1.7us apart, so the head DMA
            # has large slack on every deadline.  Bank c completes at
            # position c of the final sweep (213ns stagger); epilogues
            # alternate DVE/ACT so each engine drains 4 back-to-back.
            yps = [ps_pool.tile([P, NB], F32, tag="ps", name=f"yp0_{c}")
                   for c in range(OC)]
            for u in range(JP):
                for c in range(OC):
                    nc.tensor.matmul(yps[c][:], lhsT_dr(u, c), rhs_for(u, 0),
                                     start=(u == 0), stop=False,
                                     perf_mode=DRMODE)
            for s in range(KB):
                for c in range(OC):
                    nc.tensor.matmul(
                        yps[c][:], wt[s][:, c * P:(c + 1) * P],
                        rhs_for(JP + s, 0), start=False,
                        stop=(s == KB - 1))
                    if s == KB - 1:
                        epilogue(0, c, yps[c])

            # ---- blocks 1..7: skewed waves.  MM(unit u, out-chunk c) at
            # wave u+c; bank completions stagger ~1 wave apart.
            for n in range(1, NBC):
                yps = [ps_pool.tile([P, NB], F32, tag="ps", name=f"yp{n}_{c}")
                       for c in range(OC)]
                for wv in range(NU + OC - 1):
                    for c in range(OC):
                        u = wv - c
                        if 0 <= u < NU:
                            mm(u, c, n, yps[c])
                            if u == NU - 1:
                                epilogue(n, c, yps[c])

    nc.compile()
    return nc


_NC = None


def _get_program():
    global _NC
    if _NC is None:
        _NC = build_program()
    return _NC


def kernel(x: np.ndarray, W: np.ndarray, b: np.ndarray) -> np.ndarray:
    assert x.shape == (BATCH, IN) and W.shape == (OUT, IN) and b.shape == (OUT,)
    nc = _get_program()

    Wf = np.asarray(W, dtype=np.float32)
    sgnT = np.where(Wf >= 0, np.float32(1.0), np.float32(-1.0)).T  # [in, out]
    # st cols per j are [oh (out half)][i (k subtile)][o']
    st_pack = np.ascontiguousarray(
        sgnT[:K8].reshape(JP, 2, P, 2, OUT // 2).transpose(0, 2, 3, 1, 4)
        .reshape(JP * P, 2 * OUT)).astype(ml_dtypes.float8_e4m3)
    wt_pack = np.ascontiguousarray(sgnT[K8:]).astype(ml_dtypes.float8_e4m3)
    sc = np.maximum(np.abs(Wf).mean(axis=1), EPS).astype(np.float32)
    b32 = np.asarray(b, dtype=np.float32)
    # scb[p, c] = sc[c*128+p]; scb[p, OC+c] = b[c*128+p]
    scb = np.ascontiguousarray(
        np.concatenate([sc.reshape(OC, P).T, b32.reshape(OC, P).T],
                       axis=1).astype(np.float32))

    in_maps = []
    for c in range(N_CORES):
        xt = x[c * SHARD:(c + 1) * SHARD].T      # [in, n] view
        # x8 block-major: (j,i,p,nb,nn) -> (p, nb, j, i, nn)
        x8 = xt[:K8].astype(ml_dtypes.float8_e4m3)
        x8 = np.ascontiguousarray(
            x8.reshape(JP, 2, P, NBC, NB).transpose(2, 3, 0, 1, 4)
            .reshape(P, NBC * 2048))
        # xb: (mp,mm,p,nb,nn) -> (p, mp, nb, mm, nn)
        xb = xt[K8:].astype(ml_dtypes.bfloat16)
        xb = np.ascontiguousarray(
            xb.reshape(2, 2, P, NBC, NB).transpose(2, 0, 3, 1, 4)
            .reshape(P, 2 * NBC * 1024))
        in_maps.append({"x8": x8, "xb": xb, "st": st_pack, "wt": wt_pack,
                        "scb": scb})

    trace = bool(int(os.environ.get("BINLIN_TRACE", "0")))
    if trace:
        _install_trace_shim()
    res = run_bass_kernel_spmd(nc, in_maps, core_ids=list(range(N_CORES)),
                               trace=trace)
    if trace and res.exec_time_ns is not None:
        print(f"HW exec time: {res.exec_time_ns} ns", flush=True)

    y = np.empty((BATCH, OUT), dtype=np.float32)
    for c in range(N_CORES):
        y[c * SHARD:(c + 1) * SHARD] = res.results[c]["yt"].T.astype(np.float32)
    return y
